# revision 20
# baseline (speedup 1.0000x reference)
"""Cross-attention kernel for Trainium2, 8 NeuronCores.

Problem (hardcoded): B=4, SQ=SK=2048, DIM=1024, fp32.
    q = x1 @ Wq^T + bq ; k = x2 @ Wk^T + bk ; v = x2 @ Wv^T + bv
    out = softmax(q k^T / sqrt(D)) v

Sharding: data-parallel over batch x query-half. Core c handles batch c//2,
query rows [1024*(c%2), 1024*(c%2+1)). K/V projections are recomputed on both
cores of a batch pair (no collectives).

All matmuls run in float32r (single-pass fp32 on the PE, ~13-bit mantissa),
accumulating fp32 in PSUM. Softmax skips the max-subtraction: scores*scale
for this distribution are O(1), far from exp() overflow.

Everything is computed transposed (scores as S^T[j,i], output as out^T[e,i])
so no PE transposes are needed: PV consumes V in its natural [j,e] layout as
the stationary operand. Softmax denominators come from ones-row matmuls; the
host transposes the final [e,i] result back.
"""

import os
import numpy as np

import concourse.bass as bass
import concourse.tile as tile
from concourse import bacc, mybir
from concourse.bass_utils import run_bass_kernel_spmd

B, SQ, SK, D = 4, 2048, 2048, 1024
N_CORES = 8
QH = SQ // 2  # queries per core
SCALE = 1.0 / np.sqrt(D)

F32 = mybir.dt.float32
F32R = mybir.dt.float32r

DT = D // 128  # 8 d tiles
ET = D // 128  # 8 e tiles
NB = 4  # key blocks
JB = SK // NB  # 512 keys per block
JT = JB // 128  # 4 j tiles per block
IH = QH // 512  # 2 query column halves

_CACHE = {}

LAST_EXEC_NS = None
LAST_RESULTS = None


def _maybe_enable_trace():
    """Best-effort install of the NTFF profile hook (stripped axon client)."""
    try:
        import sys
        import types

        if "antenv.axon_hooks" not in sys.modules:
            mod = types.ModuleType("antenv.axon_hooks")
            _hook = [None]
            mod.set_axon_ntff_profile_hook = lambda h: _hook.__setitem__(0, h)
            mod.get_axon_ntff_profile_hook = lambda: _hook[0]
            import antenv

            antenv.axon_hooks = mod
            sys.modules["antenv.axon_hooks"] = mod
            from trn_agent_boot.trn_boot import _ntff_profile_via_ctypes

            mod.set_axon_ntff_profile_hook(
                _ntff_profile_via_ctypes("/opt/axon/libaxon_pjrt.so")
            )
            from concourse import bass_utils

            bass_utils.upload_artifacts = lambda tmpdir: f"local:{tmpdir}"
        return True
    except Exception:
        return False


def _build():
    nc = bacc.Bacc()

    x1T = nc.dram_tensor("x1T", [D, QH], F32R, kind="ExternalInput")
    x2T = nc.dram_tensor("x2T", [D, SK], F32R, kind="ExternalInput")
    WqT = nc.dram_tensor("WqT", [D, D], F32R, kind="ExternalInput")
    WkT = nc.dram_tensor("WkT", [D, D], F32R, kind="ExternalInput")
    WvT = nc.dram_tensor("WvT", [D, D], F32R, kind="ExternalInput")
    bqs = nc.dram_tensor("bqs", [128, 8], F32, kind="ExternalInput")
    bks = nc.dram_tensor("bks", [128, 8], F32, kind="ExternalInput")
    bvs = nc.dram_tensor("bvs", [128, 8], F32, kind="ExternalInput")
    onesc = nc.dram_tensor("onesc", [128, 1], F32R, kind="ExternalInput")
    outT = nc.dram_tensor("outT", [D, QH], F32, kind="ExternalOutput")
    rl_dram = nc.dram_tensor("rl_scratch", [1, QH], F32)

    x1T_r = x1T.rearrange("(dt p) i -> p dt i", p=128)
    x2T_r = x2T.rearrange("(dt p) j -> p dt j", p=128)

    with tile.TileContext(nc) as tc:
        with (
            tc.tile_pool(name="persist", bufs=1) as persist,
            tc.tile_pool(name="ps_proj", bufs=2, space="PSUM") as ps_proj,
            tc.tile_pool(name="ps_sc", bufs=2, space="PSUM") as ps_sc,
            tc.tile_pool(name="ps_out", bufs=2, space="PSUM") as ps_out,
            tc.tile_pool(name="ps_l", bufs=2, space="PSUM") as ps_l,
        ):
            # ---- persistent tensors; DMA issue order = need order ----
            bq_sb = persist.tile([128, 8], F32, tag="bq")
            bk_sb = persist.tile([128, 8], F32, tag="bk")
            onesc_sb = persist.tile([128, 1], F32R, tag="onesc")
            wk_sb = persist.tile([128, DT, D], F32R, tag="wk")
            wv_sb = persist.tile([128, DT, D], F32R, tag="wv")
            bvs_sb = persist.tile([128, 8], F32, tag="bvs")
            qt_sb = persist.tile([128, ET, QH], F32R, tag="qt")  # q^T [e, i]
            acc_sb = persist.tile([128, ET, QH], F32, tag="acc")  # out^T accum

            nc.sync.dma_start(out=bq_sb, in_=bqs[:, :])
            nc.sync.dma_start(out=bk_sb, in_=bks[:, :])
            nc.sync.dma_start(out=onesc_sb, in_=onesc[:, :])
            nc.sync.dma_start(out=bvs_sb, in_=bvs[:, :])

            lacc_sb = persist.tile([1, QH], F32, tag="lacc")  # softmax denoms

            # ---- phase Q: qT[e, i] = (Wq x1^T) + bq ----
            # wq comes in halves so the first matmuls start ~4 MB sooner;
            # weights stream on the SP queue, activations on the ACT queue.
            with tc.tile_pool(name="qphase", bufs=1) as qphase:
                wqr = WqT.rearrange("(dt p) e -> p dt e", p=128)
                wq_sb = qphase.tile([128, DT, D], F32R, tag="wq")
                x1h = []
                for ih in range(IH):
                    x1_sb = qphase.tile([128, DT, 512], F32R, tag=f"x1_{ih}")
                    x1h.append(x1_sb)
                nc.sync.dma_start(out=wq_sb[:, :, 0:512], in_=wqr[:, :, 0:512])
                nc.scalar.dma_start(out=x1h[0], in_=x1T_r[:, :, 0:512])
                nc.sync.dma_start(out=wq_sb[:, :, 512:1024], in_=wqr[:, :, 512:1024])
                nc.scalar.dma_start(out=x1h[1], in_=x1T_r[:, :, 512:1024])
                nc.sync.dma_start(
                    out=wk_sb, in_=WkT.rearrange("(dt p) e -> p dt e", p=128)
                )
                for eh in range(2):
                    for et in range(eh * 4, eh * 4 + 4):
                        for ih in range(IH):
                            pq = ps_proj.tile([128, 512], F32, tag="pp")
                            for d in range(DT):
                                nc.tensor.matmul(
                                    pq,
                                    wq_sb[:, d, et * 128 : (et + 1) * 128],
                                    x1h[ih][:, d, :],
                                    start=(d == 0),
                                    stop=(d == DT - 1),
                                )
                            nc.scalar.activation(
                                qt_sb[:, et, ih * 512 : (ih + 1) * 512],
                                pq,
                                mybir.ActivationFunctionType.Identity,
                                bias=bq_sb[:, et : et + 1],
                            )

            # ---- key-block loop ----
            with (
                tc.tile_pool(name="x2blk", bufs=1) as x2blk,
                tc.tile_pool(name="ktblk", bufs=2) as ktblk,
                tc.tile_pool(name="vblk", bufs=1) as vblkp,
                tc.tile_pool(name="exblk", bufs=1) as exblk,
                tc.tile_pool(name="finp", bufs=1) as finp,
            ):
                for blk in range(NB):
                    j0 = blk * JB
                    x2_sb = x2blk.tile([128, DT, JB], F32R, tag="x2")
                    nc.scalar.dma_start(out=x2_sb, in_=x2T_r[:, :, j0 : j0 + JB])
                    if blk == 0:
                        # queue remaining big DMA behind block 0's x2
                        nc.sync.dma_start(
                            out=wv_sb, in_=WvT.rearrange("(dt p) e -> p dt e", p=128)
                        )

                    # K proj: kT[e, j] for this block
                    kt_sb = ktblk.tile([128, ET, JB], F32R, tag="kt")
                    for et in range(ET):
                        pk = ps_proj.tile([128, JB], F32, tag="pp")
                        for d in range(DT):
                            nc.tensor.matmul(
                                pk,
                                wk_sb[:, d, et * 128 : (et + 1) * 128],
                                x2_sb[:, d, :],
                                start=(d == 0),
                                stop=(d == DT - 1),
                            )
                        nc.scalar.activation(
                            kt_sb[:, et, :],
                            pk,
                            mybir.ActivationFunctionType.Identity,
                            bias=bk_sb[:, et : et + 1],
                        )

                    # V proj: v[j, e] ; bias added via broadcast DVE add
                    v_sb = vblkp.tile([128, JT, D], F32R, tag="v")
                    for jt in range(JT):
                        for eh in range(2):
                            pv = ps_proj.tile([128, 512], F32, tag="pp")
                            for d in range(DT):
                                nc.tensor.matmul(
                                    pv,
                                    x2_sb[:, d, jt * 128 : (jt + 1) * 128],
                                    wv_sb[:, d, eh * 512 : (eh + 1) * 512],
                                    start=(d == 0),
                                    stop=(d == DT - 1),
                                )
                            nc.vector.tensor_copy(
                                v_sb[:, jt, eh * 512 : (eh + 1) * 512], pv
                            )

                    # scores^T + exp + denominators + PV, per query half
                    for ih in range(IH):
                        ihs = slice(ih * 512, (ih + 1) * 512)
                        ex_sb = exblk.tile([128, JT, 512], F32R, tag="ex")
                        lp_ps = ps_l.tile([1, 512], F32, tag="lp")
                        for jt in range(JT):
                            pst = ps_sc.tile([128, 512], F32, tag="sc")
                            for et in range(ET):
                                nc.tensor.matmul(
                                    pst,
                                    kt_sb[:, et, jt * 128 : (jt + 1) * 128],
                                    qt_sb[:, et, ihs],
                                    start=(et == 0),
                                    stop=(et == ET - 1),
                                )
                            nc.scalar.activation(
                                ex_sb[:, jt, :],
                                pst,
                                mybir.ActivationFunctionType.Exp,
                                scale=float(SCALE),
                            )
                            # l[i] += sum_j exp(s^T)[j, i], this block's part
                            nc.tensor.matmul(
                                lp_ps,
                                onesc_sb[:, :],
                                ex_sb[:, jt, :],
                                start=(jt == 0),
                                stop=(jt == JT - 1),
                            )
                        if blk == 0:
                            nc.vector.tensor_copy(lacc_sb[:, ihs], lp_ps)
                        else:
                            nc.vector.tensor_add(
                                lacc_sb[:, ihs], lacc_sb[:, ihs], lp_ps
                            )
                        last = blk == NB - 1
                        if last:
                            # denominators for this half are complete:
                            # rl = 1/l in place, bounce through DRAM to
                            # broadcast across partitions (overlaps PV below)
                            nc.vector.reciprocal(lacc_sb[:, ihs], lacc_sb[:, ihs])
                            nc.sync.dma_start(
                                out=rl_dram[:, ihs], in_=lacc_sb[:, ihs]
                            )
                            rlb_sb = finp.tile([128, 512], F32, tag="rlb")
                            nc.sync.dma_start(
                                out=rlb_sb,
                                in_=rl_dram[0, ihs].partition_broadcast(128),
                            )
                        # out^T[e, i] += v^T p^T  (v natural layout stationary)
                        for et in range(ET):
                            pot = ps_out.tile([128, 512], F32, tag="po")
                            for jt in range(JT):
                                nc.tensor.matmul(
                                    pot,
                                    v_sb[:, jt, et * 128 : (et + 1) * 128],
                                    ex_sb[:, jt, :],
                                    start=(jt == 0),
                                    stop=(jt == JT - 1),
                                )
                            if blk == 0:
                                nc.vector.tensor_copy(acc_sb[:, et, ihs], pot)
                            else:
                                nc.vector.tensor_add(
                                    acc_sb[:, et, ihs], acc_sb[:, et, ihs], pot
                                )
                            if last:
                                # out = acc * (1/l) + bv, in place, stream out
                                # (on gpsimd: DVE is busy draining PV psums)
                                nc.gpsimd.tensor_mul(
                                    acc_sb[:, et, ihs], acc_sb[:, et, ihs], rlb_sb
                                )
                                nc.gpsimd.tensor_scalar_add(
                                    acc_sb[:, et, ihs],
                                    acc_sb[:, et, ihs],
                                    bvs_sb[:, et : et + 1],
                                )
                                nc.scalar.dma_start(
                                    out=outT[et * 128 : (et + 1) * 128, ihs],
                                    in_=acc_sb[:, et, ihs],
                                )


    nc.compile()
    return nc


def kernel(x1, x2, Wq, bq, Wk, bk, Wv, bv):
    global LAST_EXEC_NS, LAST_RESULTS

    x1 = np.ascontiguousarray(np.asarray(x1, dtype=np.float32))
    x2 = np.ascontiguousarray(np.asarray(x2, dtype=np.float32))
    Wq = np.asarray(Wq, dtype=np.float32)
    Wk = np.asarray(Wk, dtype=np.float32)
    Wv = np.asarray(Wv, dtype=np.float32)
    bq = np.asarray(bq, dtype=np.float32)
    bk = np.asarray(bk, dtype=np.float32)
    bv = np.asarray(bv, dtype=np.float32)

    if "nc" not in _CACHE:
        _CACHE["nc"] = _build()
    nc = _CACHE["nc"]

    WqT = np.ascontiguousarray(Wq.T)
    WkT = np.ascontiguousarray(Wk.T)
    WvT = np.ascontiguousarray(Wv.T)
    bqs = np.ascontiguousarray(bq.reshape(8, 128).T)
    bks = np.ascontiguousarray(bk.reshape(8, 128).T)
    bvs = np.ascontiguousarray(bv.reshape(8, 128).T)
    onesc = np.ones((128, 1), dtype=np.float32)

    in_maps = []
    for c in range(N_CORES):
        b, h = divmod(c, 2)
        in_maps.append(
            {
                "x1T": np.ascontiguousarray(x1[b, h * QH : (h + 1) * QH, :].T),
                "x2T": np.ascontiguousarray(x2[b].T),
                "WqT": WqT,
                "WkT": WkT,
                "WvT": WvT,
                "bqs": bqs,
                "bks": bks,
                "bvs": bvs,
                "onesc": onesc,
            }
        )

    trace = os.environ.get("KERNEL_TRACE", "0") == "1" and _maybe_enable_trace()
    res = run_bass_kernel_spmd(nc, in_maps, list(range(N_CORES)), trace=trace)
    LAST_EXEC_NS = res.exec_time_ns
    LAST_RESULTS = res

    full = np.empty((B, SQ, D), dtype=np.float32)
    for c in range(N_CORES):
        b, h = divmod(c, 2)
        full[b, h * QH : (h + 1) * QH, :] = res.results[c]["outT"].T
    return full


# revision 22
# speedup vs baseline: 1.4158x; 1.4158x over previous
"""Cross-attention kernel for Trainium2, 8 NeuronCores.

Problem (hardcoded): B=4, SQ=SK=2048, DIM=1024, fp32.
    q = x1 @ Wq^T + bq ; k = x2 @ Wk^T + bk ; v = x2 @ Wv^T + bv
    out = softmax(q k^T / sqrt(D)) v

Sharding: data-parallel over batch x query-half. Core c handles batch c//2,
query rows [1024*(c%2), 1024*(c%2+1)). K/V projections are recomputed on both
cores of a batch pair (no collectives).

All matmuls run in float32r (single-pass fp32 on the PE, ~13-bit mantissa),
accumulating fp32 in PSUM. Softmax skips the max-subtraction: scores*scale
for this distribution are O(1), far from exp() overflow.

Everything is computed transposed (scores as S^T[j,i], output as out^T[e,i])
so no PE transposes are needed: PV consumes V in its natural [j,e] layout as
the stationary operand. Softmax denominators come from ones-row matmuls; the
host transposes the final [e,i] result back.
"""

import os
import numpy as np

import concourse.bass as bass
import concourse.tile as tile
from concourse import bacc, mybir
from concourse.bass_utils import run_bass_kernel_spmd

B, SQ, SK, D = 4, 2048, 2048, 1024
N_CORES = 8
QH = SQ // 2  # queries per core
SCALE = 1.0 / np.sqrt(D)

F32 = mybir.dt.float32
F32R = mybir.dt.float32r

DT = D // 128  # 8 d tiles
ET = D // 128  # 8 e tiles
NB = 4  # key blocks
JB = SK // NB  # 512 keys per block
JT = JB // 128  # 4 j tiles per block
IH = QH // 512  # 2 query column halves

_CACHE = {}

LAST_EXEC_NS = None
LAST_RESULTS = None


def _maybe_enable_trace():
    """Best-effort install of the NTFF profile hook (stripped axon client)."""
    try:
        import sys
        import types

        if "antenv.axon_hooks" not in sys.modules:
            mod = types.ModuleType("antenv.axon_hooks")
            _hook = [None]
            mod.set_axon_ntff_profile_hook = lambda h: _hook.__setitem__(0, h)
            mod.get_axon_ntff_profile_hook = lambda: _hook[0]
            import antenv

            antenv.axon_hooks = mod
            sys.modules["antenv.axon_hooks"] = mod
            from trn_agent_boot.trn_boot import _ntff_profile_via_ctypes

            mod.set_axon_ntff_profile_hook(
                _ntff_profile_via_ctypes("/opt/axon/libaxon_pjrt.so")
            )
            from concourse import bass_utils

            bass_utils.upload_artifacts = lambda tmpdir: f"local:{tmpdir}"
        return True
    except Exception:
        return False


def _build():
    nc = bacc.Bacc()

    x1T = nc.dram_tensor("x1T", [D, QH], F32R, kind="ExternalInput")
    x2T = nc.dram_tensor("x2T", [D, SK], F32R, kind="ExternalInput")
    WqT = nc.dram_tensor("WqT", [D, D], F32R, kind="ExternalInput")
    WkT = nc.dram_tensor("WkT", [D, D], F32R, kind="ExternalInput")
    WvT = nc.dram_tensor("WvT", [D, D], F32R, kind="ExternalInput")
    bqs = nc.dram_tensor("bqs", [128, 8], F32, kind="ExternalInput")
    bks = nc.dram_tensor("bks", [128, 8], F32, kind="ExternalInput")
    bvs = nc.dram_tensor("bvs", [128, 8], F32, kind="ExternalInput")
    onesc = nc.dram_tensor("onesc", [128, 1], F32R, kind="ExternalInput")
    onesr = nc.dram_tensor("onesr", [1, 128], F32R, kind="ExternalInput")
    outT = nc.dram_tensor("outT", [D, QH], F32, kind="ExternalOutput")

    x1T_r = x1T.rearrange("(dt p) i -> p dt i", p=128)
    x2T_r = x2T.rearrange("(dt p) j -> p dt j", p=128)

    with tile.TileContext(nc) as tc:
        with (
            tc.tile_pool(name="persist", bufs=1) as persist,
            tc.tile_pool(name="ps_proj", bufs=2, space="PSUM") as ps_proj,
            tc.tile_pool(name="ps_sc", bufs=2, space="PSUM") as ps_sc,
            tc.tile_pool(name="ps_out", bufs=2, space="PSUM") as ps_out,
            tc.tile_pool(name="ps_l", bufs=1, space="PSUM") as ps_l,
        ):
            # ---- persistent tensors; DMA issue order = need order ----
            bq_sb = persist.tile([128, 8], F32, tag="bq")
            bk_sb = persist.tile([128, 8], F32, tag="bk")
            onesc_sb = persist.tile([128, 1], F32R, tag="onesc")
            onesr_sb = persist.tile([1, 128], F32R, tag="onesr")
            wk_sb = persist.tile([128, DT, D], F32R, tag="wk")
            wv_sb = persist.tile([128, DT, D], F32R, tag="wv")
            bvs_sb = persist.tile([128, 8], F32, tag="bvs")
            qt_sb = persist.tile([128, ET, QH], F32R, tag="qt")  # q^T [e, i]
            acc_sb = persist.tile([128, ET, QH], F32, tag="acc")  # out^T accum

            nc.sync.dma_start(out=bq_sb, in_=bqs[:, :])
            nc.sync.dma_start(out=bk_sb, in_=bks[:, :])
            nc.sync.dma_start(out=onesc_sb, in_=onesc[:, :])
            nc.sync.dma_start(out=onesr_sb, in_=onesr[:, :])
            nc.sync.dma_start(out=bvs_sb, in_=bvs[:, :])

            lacc_sb = persist.tile([1, QH], F32, tag="lacc")  # softmax denoms

            # ---- phase Q: qT[e, i] = (Wq x1^T) + bq ----
            # wq comes in halves so the first matmuls start ~4 MB sooner;
            # weights stream on the SP queue, activations on the ACT queue.
            with tc.tile_pool(name="qphase", bufs=1) as qphase:
                wqr = WqT.rearrange("(dt p) e -> p dt e", p=128)
                wq_sb = qphase.tile([128, DT, D], F32R, tag="wq")
                x1h = []
                for ih in range(IH):
                    x1_sb = qphase.tile([128, DT, 512], F32R, tag=f"x1_{ih}")
                    x1h.append(x1_sb)
                nc.sync.dma_start(out=wq_sb[:, :, 0:512], in_=wqr[:, :, 0:512])
                nc.scalar.dma_start(out=x1h[0], in_=x1T_r[:, :, 0:512])
                nc.sync.dma_start(out=wq_sb[:, :, 512:1024], in_=wqr[:, :, 512:1024])
                nc.scalar.dma_start(out=x1h[1], in_=x1T_r[:, :, 512:1024])
                nc.sync.dma_start(
                    out=wk_sb, in_=WkT.rearrange("(dt p) e -> p dt e", p=128)
                )
                for eh in range(2):
                    for et in range(eh * 4, eh * 4 + 4):
                        for ih in range(IH):
                            pq = ps_proj.tile([128, 512], F32, tag="pp")
                            for d in range(DT):
                                nc.tensor.matmul(
                                    pq,
                                    wq_sb[:, d, et * 128 : (et + 1) * 128],
                                    x1h[ih][:, d, :],
                                    start=(d == 0),
                                    stop=(d == DT - 1),
                                )
                            nc.scalar.activation(
                                qt_sb[:, et, ih * 512 : (ih + 1) * 512],
                                pq,
                                mybir.ActivationFunctionType.Identity,
                                bias=bq_sb[:, et : et + 1],
                            )

            # ---- key-block loop ----
            with (
                tc.tile_pool(name="x2blk", bufs=1) as x2blk,
                tc.tile_pool(name="ktblk", bufs=2) as ktblk,
                tc.tile_pool(name="vblk", bufs=1) as vblkp,
                tc.tile_pool(name="exblk", bufs=1) as exblk,
                tc.tile_pool(name="finp", bufs=1) as finp,
            ):
                for blk in range(NB):
                    j0 = blk * JB
                    x2_sb = x2blk.tile([128, DT, JB], F32R, tag="x2")
                    nc.scalar.dma_start(out=x2_sb, in_=x2T_r[:, :, j0 : j0 + JB])
                    if blk == 0:
                        # queue remaining big DMA behind block 0's x2
                        nc.sync.dma_start(
                            out=wv_sb, in_=WvT.rearrange("(dt p) e -> p dt e", p=128)
                        )

                    # K proj: kT[e, j] for this block
                    kt_sb = ktblk.tile([128, ET, JB], F32R, tag="kt")
                    for et in range(ET):
                        pk = ps_proj.tile([128, JB], F32, tag="pp")
                        for d in range(DT):
                            nc.tensor.matmul(
                                pk,
                                wk_sb[:, d, et * 128 : (et + 1) * 128],
                                x2_sb[:, d, :],
                                start=(d == 0),
                                stop=(d == DT - 1),
                            )
                        nc.scalar.activation(
                            kt_sb[:, et, :],
                            pk,
                            mybir.ActivationFunctionType.Identity,
                            bias=bk_sb[:, et : et + 1],
                        )

                    # V proj: v[j, e] ; bias added via broadcast DVE add
                    v_sb = vblkp.tile([128, JT, D], F32R, tag="v")
                    for jt in range(JT):
                        for eh in range(2):
                            pv = ps_proj.tile([128, 512], F32, tag="pp")
                            for d in range(DT):
                                nc.tensor.matmul(
                                    pv,
                                    x2_sb[:, d, jt * 128 : (jt + 1) * 128],
                                    wv_sb[:, d, eh * 512 : (eh + 1) * 512],
                                    start=(d == 0),
                                    stop=(d == DT - 1),
                                )
                            nc.vector.tensor_copy(
                                v_sb[:, jt, eh * 512 : (eh + 1) * 512], pv
                            )

                    # scores^T + exp + denominators + PV, per query half
                    for ih in range(IH):
                        ihs = slice(ih * 512, (ih + 1) * 512)
                        ex_sb = exblk.tile([128, JT, 512], F32R, tag="ex")
                        lp_ps = ps_l.tile([1, 512], F32, tag="lp")
                        for jt in range(JT):
                            pst = ps_sc.tile([128, 512], F32, tag="sc")
                            for et in range(ET):
                                nc.tensor.matmul(
                                    pst,
                                    kt_sb[:, et, jt * 128 : (jt + 1) * 128],
                                    qt_sb[:, et, ihs],
                                    start=(et == 0),
                                    stop=(et == ET - 1),
                                )
                            nc.scalar.activation(
                                ex_sb[:, jt, :],
                                pst,
                                mybir.ActivationFunctionType.Exp,
                                scale=float(SCALE),
                            )
                            # l[i] += sum_j exp(s^T)[j, i], this block's part
                            nc.tensor.matmul(
                                lp_ps,
                                onesc_sb[:, :],
                                ex_sb[:, jt, :],
                                start=(jt == 0),
                                stop=(jt == JT - 1),
                            )
                        if blk == 0:
                            nc.vector.tensor_copy(lacc_sb[:, ihs], lp_ps)
                        else:
                            nc.vector.tensor_add(
                                lacc_sb[:, ihs], lacc_sb[:, ihs], lp_ps
                            )
                        last = blk == NB - 1
                        if last:
                            # denominators for this half are complete:
                            # rl = 1/l in place (as f32r), then broadcast
                            # across partitions with a K=1 ones matmul
                            rlr_sb = finp.tile([1, 512], F32R, tag="rlr")
                            with nc.allow_low_precision(
                                reason="f32r reciprocal feeds f32r broadcast"
                            ):
                                nc.vector.reciprocal(rlr_sb, lacc_sb[:, ihs])
                            rlb_ps = ps_l.tile([128, 512], F32, tag="rlb")
                            nc.tensor.matmul(
                                rlb_ps,
                                onesr_sb[:, :],
                                rlr_sb,
                                start=True,
                                stop=True,
                            )
                        # out^T[e, i] += v^T p^T  (v natural layout stationary)
                        for et in range(ET):
                            pot = ps_out.tile([128, 512], F32, tag="po")
                            for jt in range(JT):
                                nc.tensor.matmul(
                                    pot,
                                    v_sb[:, jt, et * 128 : (et + 1) * 128],
                                    ex_sb[:, jt, :],
                                    start=(jt == 0),
                                    stop=(jt == JT - 1),
                                )
                            if blk == 0:
                                nc.vector.tensor_copy(acc_sb[:, et, ihs], pot)
                            else:
                                nc.vector.tensor_add(
                                    acc_sb[:, et, ihs], acc_sb[:, et, ihs], pot
                                )
                            if last:
                                # out = acc * (1/l) + bv, in place, stream out
                                nc.vector.tensor_mul(
                                    acc_sb[:, et, ihs], acc_sb[:, et, ihs], rlb_ps
                                )
                                nc.vector.tensor_scalar_add(
                                    acc_sb[:, et, ihs],
                                    acc_sb[:, et, ihs],
                                    bvs_sb[:, et : et + 1],
                                )
                                nc.scalar.dma_start(
                                    out=outT[et * 128 : (et + 1) * 128, ihs],
                                    in_=acc_sb[:, et, ihs],
                                )


    nc.compile()
    return nc


def kernel(x1, x2, Wq, bq, Wk, bk, Wv, bv):
    global LAST_EXEC_NS, LAST_RESULTS

    x1 = np.ascontiguousarray(np.asarray(x1, dtype=np.float32))
    x2 = np.ascontiguousarray(np.asarray(x2, dtype=np.float32))
    Wq = np.asarray(Wq, dtype=np.float32)
    Wk = np.asarray(Wk, dtype=np.float32)
    Wv = np.asarray(Wv, dtype=np.float32)
    bq = np.asarray(bq, dtype=np.float32)
    bk = np.asarray(bk, dtype=np.float32)
    bv = np.asarray(bv, dtype=np.float32)

    if "nc" not in _CACHE:
        _CACHE["nc"] = _build()
    nc = _CACHE["nc"]

    WqT = np.ascontiguousarray(Wq.T)
    WkT = np.ascontiguousarray(Wk.T)
    WvT = np.ascontiguousarray(Wv.T)
    bqs = np.ascontiguousarray(bq.reshape(8, 128).T)
    bks = np.ascontiguousarray(bk.reshape(8, 128).T)
    bvs = np.ascontiguousarray(bv.reshape(8, 128).T)
    onesc = np.ones((128, 1), dtype=np.float32)
    onesr = np.ones((1, 128), dtype=np.float32)

    in_maps = []
    for c in range(N_CORES):
        b, h = divmod(c, 2)
        in_maps.append(
            {
                "x1T": np.ascontiguousarray(x1[b, h * QH : (h + 1) * QH, :].T),
                "x2T": np.ascontiguousarray(x2[b].T),
                "WqT": WqT,
                "WkT": WkT,
                "WvT": WvT,
                "bqs": bqs,
                "bks": bks,
                "bvs": bvs,
                "onesc": onesc,
                "onesr": onesr,
            }
        )

    trace = os.environ.get("KERNEL_TRACE", "0") == "1" and _maybe_enable_trace()
    res = run_bass_kernel_spmd(nc, in_maps, list(range(N_CORES)), trace=trace)
    LAST_EXEC_NS = res.exec_time_ns
    LAST_RESULTS = res

    full = np.empty((B, SQ, D), dtype=np.float32)
    for c in range(N_CORES):
        b, h = divmod(c, 2)
        full[b, h * QH : (h + 1) * QH, :] = res.results[c]["outT"].T
    return full
